# revision 1
# baseline (speedup 1.0000x reference)
# Trainium2 Bass kernel for single-head causal attention
#   q = x@Wq, k = x@Wk, v = x@Wv   (x [B,T,C], W* [C,H])
#   out = softmax(mask(q k^T / sqrt(C))) @ v
# B=512, T=142, C=512, H=64.  Data-parallel over B across 8 NeuronCores.
#
# Device-side layout strategy (per core, 64 batches = 9088 tokens):
#  - host feeds x^T  [4,128,9088]  (contraction dim C on partitions)
#  - qT = Wq-stationary matmuls -> psum [128,*] rows 0:64 (Wq zero-padded)
#  - k,v packed:  [Wk|Wv] stationary -> psum rows 0:64 = kT, 64:128 = vT
#  - scores weiT[s,t] = kT-stationary matmul; causal mask added via one
#    identity-stationary matmul accumulating a mask tile into PSUM
#  - exp on ScalarE (scale=C^-0.5 fused), result bf16 in SBUF
#  - v natural [s,h] via identity-matmul transpose of vT
#  - AV: exp-scores stationary, rhs = [v | ones] -> out [t, 65] where
#    col 64 = softmax denominator; division happens on host (glue).
# Groups of 3 batches; group PAIRS share one x DMA and one output DMA
# to keep the SP sequencer / HWDGE ring off the critical path.
import os

import numpy as np
import ml_dtypes

B, T, C, H = 512, 142, 512, 64
NCORES = 8
NB = B // NCORES            # 64 batches per core
NT = NB * T                 # 9088 tokens per core
GB = 3                      # batches per processing group
NG = (NB + GB - 1) // GB    # 22 groups (21 full + 1 single)
SCALE = float(C) ** -0.5
NEG = -1e30
TW = 65                     # out block width: H + 1 denominator column

_CACHE = {}


def _groups():
    return [(g * GB, min(GB, NB - g * GB)) for g in range(NG)]


def _build_nc():
    import concourse.bacc as bacc
    import concourse.mybir as mybir
    from concourse.tile import TileContext

    fp32 = mybir.dt.float32
    bf16 = mybir.dt.bfloat16
    Exp = mybir.ActivationFunctionType.Exp

    nc = bacc.Bacc(
        "TRN2",
        target_bir_lowering=False,
        debug=False,
        enable_asserts=False,
        num_devices=NCORES,
    )

    xt = nc.dram_tensor("xt", [4, 128, NT], bf16, kind="ExternalInput").ap()
    # all 8 weight chunks in one tensor: [Wq|0] chunks then [Wk|Wv] chunks
    wts = nc.dram_tensor("wts", [8, 128, 128], bf16, kind="ExternalInput").ap()
    # constants blob: cols 0:426 mask3, 426:468 mskt3 (rows 0:14),
    # 468:596 identity128, 596:660 idhi
    cst = nc.dram_tensor("cst", [128, 660], bf16, kind="ExternalInput").ap()
    om = nc.dram_tensor("om", [NG, 128, GB * TW], fp32, kind="ExternalOutput").ap()
    ot = nc.dram_tensor("ot", [NG, 14, GB * TW], fp32, kind="ExternalOutput").ap()

    GT = GB * T           # 426 token columns per full group
    TAIL0 = GT            # col offset of tail score blocks in psc
    groups = _groups()
    pairs = [(2 * p, min(2, NG - 2 * p)) for p in range((NG + 1) // 2)]

    with TileContext(nc) as tc:
        with (
            tc.tile_pool(name="const", bufs=1) as cpool,
            tc.tile_pool(name="xtp", bufs=2) as xpool,
            tc.tile_pool(name="work", bufs=3) as wpool,
            tc.tile_pool(name="psum", bufs=1, space="PSUM") as ppool,
        ):
            wts_sb = cpool.tile([128, 8 * 128], bf16)
            cst_sb = cpool.tile([128, 660], bf16)
            nc.sync.dma_start(
                out=wts_sb.rearrange("p (c w) -> p c w", c=8),
                in_=wts.rearrange("c p w -> p c w"))
            nc.sync.dma_start(out=cst_sb[:, :], in_=cst)

            def wq_c(c):
                return wts_sb[:, c * 128:(c + 1) * 128]

            def wkv_c(c):
                return wts_sb[:, 512 + c * 128:512 + (c + 1) * 128]

            msk3_sb = cst_sb[:, 0:426]
            mskt3_sb = cst_sb[0:14, 426:468]
            iden_sb = cst_sb[:, 468:596]
            idhi_sb = cst_sb[:, 596:660]

            for g0, np_ in pairs:
                pg = groups[g0:g0 + np_]
                gtp = sum(nb for _, nb in pg) * T
                t0 = pg[0][0] * T

                xt_t = xpool.tile([128, 4 * 2 * GT], bf16, tag="xt")
                if g0 == 0:
                    for c in range(4):
                        nc.sync.dma_start(
                            out=xt_t[:, c * gtp:(c + 1) * gtp],
                            in_=xt[c, :, t0:t0 + gtp],
                        )
                else:
                    nc.sync.dma_start(
                        out=xt_t[:, 0:4 * gtp].rearrange("p (c t) -> p c t", c=4),
                        in_=xt[:, :, t0:t0 + gtp].rearrange("c p t -> p c t"),
                    )

                o_sb = wpool.tile([128, 2 * GB * TW], fp32, tag="o")
                o2_sb = wpool.tile([14, 2 * GB * TW], fp32, tag="o2")

                for s, (b0, nb) in enumerate(pg):
                    gt = nb * T
                    off = (b0 * T) - t0          # token offset within pair tile

                    # ---- QKV projections ----
                    pq = ppool.tile([128, GT], fp32, tag="pq", bufs=2)
                    pkv = ppool.tile([128, GT], fp32, tag="pkv", bufs=2)
                    # kv first: the ACT kv-copy (scores' stationary operand)
                    # then overlaps the q matmuls on PE
                    for c in range(4):
                        rhs = xt_t[:, c * gtp + off:c * gtp + off + gt]
                        nc.tensor.matmul(
                            pkv[:, :gt], lhsT=wkv_c(c), rhs=rhs,
                            start=(c == 0), stop=(c == 3),
                        )
                    q_sb = wpool.tile([64, GT], bf16, tag="q")
                    kv_sb = wpool.tile([128, GT], bf16, tag="kv")
                    nc.scalar.copy(kv_sb[:, :gt], pkv[:, :gt])
                    for c in range(4):
                        rhs = xt_t[:, c * gtp + off:c * gtp + off + gt]
                        nc.tensor.matmul(
                            pq[:, :gt], lhsT=wq_c(c), rhs=rhs,
                            start=(c == 0), stop=(c == 3),
                        )
                    nc.vector.tensor_copy(q_sb[:, :gt], pq[0:64, :gt])

                    # ---- scores weiT[s,t] + causal mask ----
                    psc = ppool.tile([128, GT + GB * 14], fp32, tag="psc", bufs=2)
                    for j in range(nb):
                        cl = j * T
                        nc.tensor.matmul(
                            psc[:, cl:cl + T],
                            lhsT=kv_sb[0:64, cl:cl + 128],
                            rhs=q_sb[0:64, cl:cl + T],
                            start=True, stop=False,
                        )
                        nc.tensor.matmul(
                            psc[:, cl:cl + T],
                            lhsT=iden_sb,
                            rhs=msk3_sb[:, 0:T],
                            start=False, stop=True,
                        )
                        tco = TAIL0 + j * 14
                        nc.tensor.matmul(
                            psc[0:14, tco:tco + 14],
                            lhsT=kv_sb[0:64, cl + 128:cl + T],
                            rhs=q_sb[0:64, cl + 128:cl + T],
                            start=True, stop=False,
                        )
                        nc.tensor.matmul(
                            psc[0:14, tco:tco + 14],
                            lhsT=iden_sb[0:14, 0:14],
                            rhs=mskt3_sb[:, 0:14],
                            start=False, stop=True,
                        )

                    exp_sb = wpool.tile([128, GT + GB * 14], bf16, tag="exp")
                    nc.scalar.activation(
                        exp_sb[:, 0:gt], psc[:, 0:gt], Exp, scale=SCALE)
                    nc.scalar.activation(
                        exp_sb[0:14, TAIL0:TAIL0 + nb * 14],
                        psc[0:14, TAIL0:TAIL0 + nb * 14],
                        Exp, scale=SCALE,
                    )

                    # ---- v natural via identity-matmul transpose ----
                    pvt = ppool.tile([128, GB * 128], fp32, tag="pvt")
                    for j in range(nb):
                        cl = j * T
                        nc.tensor.matmul(
                            pvt[:, j * 64:(j + 1) * 64],
                            lhsT=kv_sb[64:128, cl:cl + 128],
                            rhs=idhi_sb[64:128, :],
                            start=True, stop=True,
                        )
                        nc.tensor.matmul(
                            pvt[0:14, GB * 64 + j * 64:GB * 64 + (j + 1) * 64],
                            lhsT=kv_sb[64:128, cl + 128:cl + T],
                            rhs=idhi_sb[64:128, :],
                            start=True, stop=True,
                        )
                    vex_sb = wpool.tile([128, GB * TW], bf16, tag="vex")
                    vext_sb = wpool.tile([14, GB * TW], bf16, tag="vext")
                    nc.vector.tensor_copy(
                        vex_sb.rearrange("p (b h) -> p b h", h=TW)[:, 0:nb, 0:64],
                        pvt[:, 0:nb * 64].rearrange("p (b h) -> p b h", h=64),
                    )
                    nc.vector.tensor_copy(
                        vext_sb.rearrange("p (b h) -> p b h", h=TW)[:, 0:nb, 0:64],
                        pvt[0:14, GB * 64:GB * 64 + nb * 64].rearrange(
                            "p (b h) -> p b h", h=64),
                    )
                    nc.vector.memset(
                        vex_sb.rearrange("p (b h) -> p b h", h=TW)[:, 0:nb, 64:65],
                        1.0)
                    nc.vector.memset(
                        vext_sb.rearrange("p (b h) -> p b h", h=TW)[:, 0:nb, 64:65],
                        1.0)

                    # ---- AV: out[t,0:64] = sum_s P^T[s,t] v[s,:], col64=denom ----
                    pout = ppool.tile([128, 2 * GB * TW], fp32, tag="pout")
                    TL = GB * TW
                    for j in range(nb):
                        cl = j * T
                        nc.tensor.matmul(
                            pout[:, j * TW:(j + 1) * TW],
                            lhsT=exp_sb[:, cl:cl + 128],
                            rhs=vex_sb[:, j * TW:(j + 1) * TW],
                            start=True, stop=True,
                        )
                        nc.tensor.matmul(
                            pout[0:14, TL + j * TW:TL + (j + 1) * TW],
                            lhsT=exp_sb[:, cl + 128:cl + T],
                            rhs=vex_sb[:, j * TW:(j + 1) * TW],
                            start=True, stop=False,
                        )
                        nc.tensor.matmul(
                            pout[0:14, TL + j * TW:TL + (j + 1) * TW],
                            lhsT=exp_sb[0:14, TAIL0 + j * 14:TAIL0 + (j + 1) * 14],
                            rhs=vext_sb[0:14, j * TW:(j + 1) * TW],
                            start=False, stop=True,
                        )

                    oc = s * GB * TW
                    nc.scalar.copy(
                        o_sb[:, oc:oc + nb * TW], pout[:, 0:nb * TW])
                    nc.vector.tensor_copy(
                        o2_sb[0:14, oc:oc + nb * TW],
                        pout[0:14, TL:TL + nb * TW])

                # ---- batched output stores (one per pair per tensor) ----
                last_nb = pg[-1][1]
                if np_ == 2 and last_nb == GB:
                    nc.gpsimd.dma_start(
                        out=om[g0:g0 + 2].rearrange("g p c -> p g c"),
                        in_=o_sb.rearrange("p (g c) -> p g c", g=2),
                    )
                    nc.gpsimd.dma_start(
                        out=ot[g0:g0 + 2].rearrange("g p c -> p g c"),
                        in_=o2_sb.rearrange("p (g c) -> p g c", g=2),
                    )
                else:
                    for s, (b0, nb) in enumerate(pg):
                        oc = s * GB * TW
                        nc.gpsimd.dma_start(
                            out=om[g0 + s, :, 0:nb * TW],
                            in_=o_sb[:, oc:oc + nb * TW])
                        nc.gpsimd.dma_start(
                            out=ot[g0 + s, :, 0:nb * TW],
                            in_=o2_sb[0:14, oc:oc + nb * TW])

    nc.compile()
    return nc


def _prep_shared(Wq, Wk, Wv):
    bf16 = ml_dtypes.bfloat16
    wq_pad = np.concatenate([Wq, np.zeros((C, H), np.float32)], axis=1)
    wkv = np.concatenate([Wk, Wv], axis=1)
    wts_np = np.concatenate(
        [
            np.ascontiguousarray(wq_pad.reshape(4, 128, 128)),
            np.ascontiguousarray(wkv.reshape(4, 128, 128)),
        ],
        axis=0,
    ).astype(bf16)

    s = np.arange(128)[:, None]
    t = np.arange(T)[None, :]
    msk = np.where(s <= t, 0.0, NEG).astype(np.float32)
    i = np.arange(14)[:, None]
    j = np.arange(14)[None, :]
    mskt = np.where(i <= j, 0.0, NEG).astype(np.float32)
    idhi = np.zeros((128, 64), np.float32)
    idhi[64 + np.arange(64), np.arange(64)] = 1.0

    cst = np.zeros((128, 660), np.float32)
    cst[:, 0:426] = np.tile(msk, (1, 3))
    cst[0:14, 426:468] = np.tile(mskt, (1, 3))
    cst[:, 468:596] = np.eye(128, dtype=np.float32)
    cst[:, 596:660] = idhi
    return dict(wts=wts_np, cst=cst.astype(bf16))


def _prep_core_xt(x_core):
    # x_core [NB, T, C] fp32 -> [4, 128, NT] bf16 (x^T, C on partitions)
    xt = x_core.reshape(NT, C).T            # [C, NT] view
    xt = np.ascontiguousarray(xt).reshape(4, 128, NT)
    return xt.astype(ml_dtypes.bfloat16)


def _assemble_core(om_np, ot_np):
    # om [NG, 128, GB*TW], ot [NG, 14, GB*TW] -> [NB, T, H] normalized
    bm = om_np.reshape(NG, 128, GB, TW).transpose(0, 2, 1, 3).reshape(NG * GB, 128, TW)
    bt = ot_np.reshape(NG, 14, GB, TW).transpose(0, 2, 1, 3).reshape(NG * GB, 14, TW)
    bm = bm[:NB].astype(np.float32)
    bt = bt[:NB].astype(np.float32)
    full = np.concatenate([bm, bt], axis=1)         # [NB, 142, TW]
    return full[:, :, 0:H] / full[:, :, H:H + 1]


def kernel(**inputs):
    x = np.asarray(inputs["x"], dtype=np.float32)
    Wq = np.asarray(inputs["Wq"], dtype=np.float32)
    Wk = np.asarray(inputs["Wk"], dtype=np.float32)
    Wv = np.asarray(inputs["Wv"], dtype=np.float32)

    from concourse.bass_utils import run_bass_kernel_spmd

    if "nc" not in _CACHE:
        _CACHE["nc"] = _build_nc()
    nc = _CACHE["nc"]

    shared = _prep_shared(Wq, Wk, Wv)
    in_maps = []
    for core in range(NCORES):
        m = dict(shared)
        m["xt"] = _prep_core_xt(x[core * NB:(core + 1) * NB])
        in_maps.append(m)

    trace = bool(int(os.environ.get("TRN_KERNEL_TRACE", "0")))
    res = run_bass_kernel_spmd(
        nc, in_maps, core_ids=list(range(NCORES)), trace=trace,
    )
    _CACHE["last_result"] = res

    outs = []
    for core in range(NCORES):
        r = res.results[core]
        outs.append(_assemble_core(np.asarray(r["om"]), np.asarray(r["ot"])))
    return np.concatenate(outs, axis=0).astype(np.float32)



# revision 8
# speedup vs baseline: 1.1274x; 1.1274x over previous
# Trainium2 Bass kernel for single-head causal attention
#   q = x@Wq, k = x@Wk, v = x@Wv   (x [B,T,C], W* [C,H])
#   out = softmax(mask(q k^T / sqrt(C))) @ v
# B=512, T=142, C=512, H=64.  Data-parallel over B across 8 NeuronCores.
#
# Strategy (per core, 64 batches = 9088 tokens):
#  - split-fp8 projections: x and 32*W are each split into hi+lo e4m3
#    planes; every projection is 3 DoubleRow terms (xh*Wh + xl*Wh + xh*Wl),
#    contracting 256 rows per pass at 0.5 PE-cycles/column.
#  - QK pass: [Wq|Wk] packed stationary -> psum [128, tokens] (q rows 0:64,
#    k rows 64:128); copied to SBUF bf16. The q half is then shifted to
#    partitions 64:128 via SBUF->SBUF DMA so the scores matmul has both
#    operands at base partition 64 (HW requires equal base partitions).
#  - V pass: x-stationary per batch -> v in natural [token, 64] layout
#    (no transpose), plus a ones column for the softmax denominator.
#  - scores: kT-stationary matmuls + causal mask added via an
#    identity-stationary matmul into PSUM; exp on ScalarE (scale fused).
#  - AV with out = [65 features, tokens]: 156 PE cycles/batch; row 64 is
#    the denominator. Division (and /32 weight-scale) happens on host.
#  - streamed in 11 token-chunks (2 groups of 3 batches each): input DMA,
#    QK proj, q-shift, V proj for chunk c overlap attention of chunk c-1.
import os

import numpy as np
import ml_dtypes

B, T, C, H = 512, 142, 512, 64
NCORES = 8
NB = B // NCORES            # 64 batches per core
NT = NB * T                 # 9088 tokens per core
GB = 3                      # batches per group
NG = (NB + GB - 1) // GB    # 22 groups (21 full + 1 single)
GT = GB * T                 # 426
WS = 32.0                   # weight pre-scale (power of two: exact in bf16)
EXP_SCALE = float(C) ** -0.5 / (WS * WS)
NEG = -1e30

# projection terms as (w_plane, x_plane); 0 = hi, 1 = lo
QK_TERMS = [(0, 0), (0, 1), (1, 0)]
V_TERMS = [(0, 0), (0, 1), (1, 0)]

_CACHE = {}


def _groups():
    return [(g * GB, min(GB, NB - g * GB)) for g in range(NG)]


def _chunks():
    # chunks of 2 groups: 10 x 852 tokens + 1 x 568 tokens
    return [(2 * c, min(2, NG - 2 * c)) for c in range((NG + 1) // 2)]


def _build_nc():
    import concourse.bacc as bacc
    import concourse.mybir as mybir
    from concourse.tile import TileContext

    fp32 = mybir.dt.float32
    bf16 = mybir.dt.bfloat16
    f8e4 = mybir.dt.float8e4
    Exp = mybir.ActivationFunctionType.Exp
    DR = mybir.MatmulPerfMode.DoubleRow

    nc = bacc.Bacc(
        "TRN2",
        target_bir_lowering=False,
        debug=False,
        enable_asserts=False,
        num_devices=NCORES,
    )

    xt8 = nc.dram_tensor("xt8", [2, 4, 128, NT], f8e4, kind="ExternalInput").ap()
    wqk = nc.dram_tensor("wqk", [128, 8, 128], f8e4, kind="ExternalInput").ap()
    wv8 = nc.dram_tensor("wv8", [128, 8, 64], f8e4, kind="ExternalInput").ap()
    # cst: cols 0:128 causal mask, 128:256 identity128, 256:270 mask14 (rows 0:14)
    cst = nc.dram_tensor("cst", [128, 270], bf16, kind="ExternalInput").ap()
    om = nc.dram_tensor("om", [NG, 65, GT], bf16, kind="ExternalOutput").ap()

    groups = _groups()
    chunks = _chunks()

    with TileContext(nc) as tc:
        with (
            tc.tile_pool(name="const", bufs=1) as cpool,
            tc.tile_pool(name="xin", bufs=3) as xpool,
            tc.tile_pool(name="work", bufs=2) as wpool,
            tc.tile_pool(name="psum", bufs=1, space="PSUM") as ppool,
        ):
            w_sb = cpool.tile([128, 8, 128], f8e4)
            wv_sb = cpool.tile([128, 8, 64], f8e4)
            cst_sb = cpool.tile([128, 270], bf16)
            qk_sb = cpool.tile([128, NT], bf16)
            qs_sb = cpool.tile([128, NT], bf16)
            nc.sync.dma_start(out=w_sb, in_=wqk)
            nc.sync.dma_start(out=wv_sb, in_=wv8)
            nc.sync.dma_start(out=cst_sb, in_=cst)

            iden = cst_sb[:, 128:256]
            mask = cst_sb[:, 0:128]
            iden14 = cst_sb[0:14, 128:142]
            mskt = cst_sb[0:14, 256:270]

            gstate = {}

            def attention(g):
                b0, nb, vex = gstate.pop(g)
                gc0 = b0 * T
                psc = ppool.tile([128, 468], fp32, tag="psc", bufs=2)
                for j in range(nb):
                    cl = j * T
                    gj = gc0 + cl
                    nc.tensor.matmul(
                        psc[:, cl:cl + 128],
                        lhsT=qk_sb[64:128, gj:gj + 128],
                        rhs=qs_sb[64:128, gj:gj + 128],
                        start=True, stop=False,
                    )
                    nc.tensor.matmul(
                        psc[:, cl:cl + 128], lhsT=iden, rhs=mask,
                        start=False, stop=True,
                    )
                    nc.tensor.matmul(
                        psc[:, cl + 128:cl + T],
                        lhsT=qk_sb[64:128, gj:gj + 128],
                        rhs=qs_sb[64:128, gj + 128:gj + T],
                        start=True, stop=True,
                    )
                    tc0 = nb * T + j * 14
                    nc.tensor.matmul(
                        psc[0:14, tc0:tc0 + 14],
                        lhsT=qk_sb[64:128, gj + 128:gj + T],
                        rhs=qs_sb[64:128, gj + 128:gj + T],
                        start=True, stop=False,
                    )
                    nc.tensor.matmul(
                        psc[0:14, tc0:tc0 + 14], lhsT=iden14, rhs=mskt,
                        start=False, stop=True,
                    )
                exp_t = wpool.tile([128, 468], bf16, tag="exp", bufs=2)
                nc.scalar.activation(
                    exp_t[:, 0:nb * 156], psc[:, 0:nb * 156], Exp,
                    scale=EXP_SCALE,
                )
                po = ppool.tile([65, GT], fp32, tag="po", bufs=2)
                for j in range(nb):
                    cl = j * T
                    nc.tensor.matmul(
                        po[:, cl:cl + 128],
                        lhsT=vex[:, j, :], rhs=exp_t[:, cl:cl + 128],
                        start=True, stop=True,
                    )
                    nc.tensor.matmul(
                        po[:, cl + 128:cl + T],
                        lhsT=vex[:, j, :], rhs=exp_t[:, cl + 128:cl + T],
                        start=True, stop=False,
                    )
                    nc.tensor.matmul(
                        po[:, cl + 128:cl + T],
                        lhsT=vex[0:14, GB + j, :],
                        rhs=exp_t[0:14, nb * T + j * 14:nb * T + (j + 1) * 14],
                        start=False, stop=True,
                    )
                return po, nb

            pend_out = []   # [(g, po, nb)]

            def flush_out(c):
                # copy pending AV psums to sbuf and DMA per chunk
                g0 = 2 * c
                osb = wpool.tile([65, 2, GT], bf16, tag="osb", bufs=2)
                for s, (g, po, nb) in enumerate(pend_out):
                    h = (nb * T) // 2
                    nc.vector.tensor_copy(osb[:, s, 0:h], po[:, 0:h])
                    nc.scalar.copy(osb[:, s, h:nb * T], po[:, h:nb * T])
                ng = len(pend_out)
                pend_out.clear()
                nc.scalar.dma_start(
                    out=om[g0:g0 + ng].rearrange("g p c -> p g c"),
                    in_=osb[:, 0:ng, :],
                )

            for c, (gc0_idx, ngrp) in enumerate(chunks):
                cgroups = groups[gc0_idx:gc0_idx + ngrp]
                t0 = cgroups[0][0] * T
                tlen = sum(nb for _, nb in cgroups) * T

                xt_t = xpool.tile([128, 2, 4, 864], f8e4, tag="x")
                nc.sync.dma_start(
                    out=xt_t[:, :, :, 0:tlen],
                    in_=xt8[:, :, :, t0:t0 + tlen].rearrange(
                        "a c p t -> p a c t"),
                )

                # ---- QK projection per group ----
                for gi, (b0, nb) in enumerate(cgroups):
                    g = gc0_idx + gi
                    gt = nb * T
                    off = b0 * T - t0
                    pq = ppool.tile([128, GT], fp32, tag="pq", bufs=2)
                    nmm = len(QK_TERMS) * 2
                    i = 0
                    for (wp, xp) in QK_TERMS:
                        for kp in range(2):
                            wi = wp * 4 + kp * 2
                            nc.tensor.matmul(
                                pq[:, 0:gt],
                                lhsT=w_sb[:, wi:wi + 2, :],
                                rhs=xt_t[:, xp, 2 * kp:2 * kp + 2,
                                         off:off + gt],
                                start=(i == 0), stop=(i == nmm - 1),
                                perf_mode=DR,
                            )
                            i += 1
                    gcol = b0 * T
                    if g % 2 == 0:
                        nc.vector.tensor_copy(
                            qk_sb[:, gcol:gcol + gt], pq[:, 0:gt])
                    else:
                        nc.scalar.copy(
                            qk_sb[:, gcol:gcol + gt], pq[:, 0:gt])

                # ---- q shift to partitions 64:128 (SBUF->SBUF DMA) ----
                nc.scalar.dma_start(
                    out=qs_sb[64:128, t0:t0 + tlen],
                    in_=qk_sb[0:64, t0:t0 + tlen],
                )

                # ---- V projection per group (natural layout) ----
                for gi, (b0, nb) in enumerate(cgroups):
                    g = gc0_idx + gi
                    off = b0 * T - t0
                    vm = ppool.tile([128, 384], fp32, tag="vm", bufs=2)
                    nmm = len(V_TERMS) * 2
                    for j in range(nb):
                        tb = off + j * T
                        i = 0
                        for (wp, xp) in V_TERMS:
                            for kp in range(2):
                                wi = wp * 4 + kp * 2
                                nc.tensor.matmul(
                                    vm[:, j * 64:(j + 1) * 64],
                                    lhsT=xt_t[:, xp, 2 * kp:2 * kp + 2,
                                              tb:tb + 128],
                                    rhs=wv_sb[:, wi:wi + 2, :],
                                    start=(i == 0), stop=(i == nmm - 1),
                                    perf_mode=DR,
                                )
                                i += 1
                        i = 0
                        for (wp, xp) in V_TERMS:
                            for kp in range(2):
                                wi = wp * 4 + kp * 2
                                nc.tensor.matmul(
                                    vm[0:14, 192 + j * 64:192 + (j + 1) * 64],
                                    lhsT=xt_t[:, xp, 2 * kp:2 * kp + 2,
                                              tb + 128:tb + T],
                                    rhs=wv_sb[:, wi:wi + 2, :],
                                    start=(i == 0), stop=(i == nmm - 1),
                                    perf_mode=DR,
                                )
                                i += 1
                    # vex slots 0:3 = v main [128 tok, 64]; slots 3:6 = v tail
                    # (rows 0:14 valid; rows 14:128 stale psum, never read)
                    vex = wpool.tile([128, 2 * GB, 65], bf16, tag="vex",
                                     bufs=4)
                    if nb == GB:
                        nc.vector.tensor_copy(
                            vex[:, 0:6, 0:64],
                            vm[:, 0:384].rearrange("p (b h) -> p b h", h=64),
                        )
                    else:
                        nc.vector.tensor_copy(
                            vex[:, 0:nb, 0:64],
                            vm[:, 0:nb * 64].rearrange(
                                "p (b h) -> p b h", h=64),
                        )
                        nc.vector.tensor_copy(
                            vex[0:14, GB:GB + nb, 0:64],
                            vm[0:14, 192:192 + nb * 64].rearrange(
                                "p (b h) -> p b h", h=64),
                        )
                    nc.gpsimd.memset(vex[:, 0:nb, 64:65], 1.0)
                    nc.gpsimd.memset(vex[0:14, GB:GB + nb, 64:65], 1.0)
                    gstate[g] = (b0, nb, vex)

                # ---- attention for previous chunk's groups ----
                if c > 0:
                    pg0, png = chunks[c - 1]
                    for g in range(pg0, pg0 + png):
                        po, nb = attention(g)
                        pend_out.append((g, po, nb))
                    flush_out(c - 1)

            # final chunk's attention
            pg0, png = chunks[-1]
            for g in range(pg0, pg0 + png):
                po, nb = attention(g)
                pend_out.append((g, po, nb))
            flush_out(len(chunks) - 1)

    nc.compile()
    return nc


def _prep_shared(Wq, Wk, Wv):
    f8 = ml_dtypes.float8_e4m3fn
    bf = ml_dtypes.bfloat16

    def split(W):
        Wp = (W * WS).astype(np.float32)
        hi = Wp.astype(f8)
        lo = (Wp - hi.astype(np.float32)).astype(f8)
        return hi, lo

    qk = np.concatenate([Wq, Wk], axis=1)            # [512, 128]
    qk_hi, qk_lo = split(qk)
    v_hi, v_lo = split(Wv)

    def pack(hi, lo, m):
        # [128, 8, m]: index = plane*4 + chunk (chunk = 2*kp + sub)
        a = np.concatenate(
            [hi.reshape(4, 128, m), lo.reshape(4, 128, m)], axis=0)
        return np.ascontiguousarray(a.transpose(1, 0, 2))

    s = np.arange(128)[:, None]
    t = np.arange(128)[None, :]
    msk = np.where(s <= t, 0.0, NEG).astype(np.float32)
    i14 = np.arange(14)[:, None]
    j14 = np.arange(14)[None, :]
    mskt = np.where(i14 <= j14, 0.0, NEG).astype(np.float32)
    cst = np.zeros((128, 270), np.float32)
    cst[:, 0:128] = msk
    cst[:, 128:256] = np.eye(128, dtype=np.float32)
    cst[0:14, 256:270] = mskt

    return dict(
        wqk=pack(qk_hi, qk_lo, 128),
        wv8=pack(v_hi, v_lo, 64),
        cst=cst.astype(bf),
    )


def _prep_core_x(x_core):
    # x_core [NB, T, C] fp32 -> [2, 4, 128, NT] e4m3 (hi, lo planes of x^T)
    f8 = ml_dtypes.float8_e4m3fn
    xt = np.ascontiguousarray(x_core.reshape(NT, C).T)   # [C, NT]
    hi = xt.astype(f8)
    lo = (xt - hi.astype(np.float32)).astype(f8)
    return np.stack(
        [hi.reshape(4, 128, NT), lo.reshape(4, 128, NT)], axis=0)


def _assemble_core(om_np):
    # om [NG, 65, GT] bf16 -> [NB, T, H]
    om = om_np.astype(np.float32)
    out = np.empty((NB, T, H), np.float32)
    for g, (b0, nb) in enumerate(_groups()):
        for j in range(nb):
            blk = om[g, :, j * T:(j + 1) * T]       # [65, T]
            out[b0 + j] = (blk[0:64] / blk[64:65]).T / WS
    return out


def kernel(**inputs):
    x = np.asarray(inputs["x"], dtype=np.float32)
    Wq = np.asarray(inputs["Wq"], dtype=np.float32)
    Wk = np.asarray(inputs["Wk"], dtype=np.float32)
    Wv = np.asarray(inputs["Wv"], dtype=np.float32)

    from concourse.bass_utils import run_bass_kernel_spmd

    if "nc" not in _CACHE:
        _CACHE["nc"] = _build_nc()
    nc = _CACHE["nc"]

    shared = _prep_shared(Wq, Wk, Wv)
    in_maps = []
    for core in range(NCORES):
        m = dict(shared)
        m["xt8"] = _prep_core_x(x[core * NB:(core + 1) * NB])
        in_maps.append(m)

    trace = bool(int(os.environ.get("TRN_KERNEL_TRACE", "0")))
    res = run_bass_kernel_spmd(
        nc, in_maps, core_ids=list(range(NCORES)), trace=trace,
    )
    _CACHE["last_result"] = res

    outs = []
    for core in range(NCORES):
        outs.append(_assemble_core(np.asarray(res.results[core]["om"])))
    return np.concatenate(outs, axis=0).astype(np.float32)
